# revision 3
# baseline (speedup 1.0000x reference)
"""Trainium2 Bass kernel for conv-stack + NetVLAD pooling + linear head.

Pure data parallel: 32 images sharded 4-per-core across 8 NeuronCores.

Phase A: host pre-interleaves x into conv1 band-tile layout (partition-
major, bf16, zero-padded halos) so each image loads with ONE fat 2D DMA
[128 x 17.5KB] at ~286 GB/s instead of 51 thin strided DMAs at ~43 GB/s.

Phase C: conv2/y1/V in bf16. NetVLAD restructured: per tile ONE matmul
with stationary V and rhs [A|S] yields logits AND pair-summed xf^T in one
pass (no PE transposes, no per-tile DVE soup); softmax batched over all
32 tiles; gram flipped (stationary = 16-col asm slice, moving = xftones)
accumulating [16, 68] in PSUM with diagonal-block extraction in the
finale.
"""
import sys

sys.path.insert(0, "/opt/trn_rl_repo")

import ml_dtypes
import numpy as np
import concourse.bacc as bacc
import concourse.tile as tile
from concourse import mybir
from concourse.bass_utils import run_bass_kernel_spmd

F32 = mybir.dt.float32
F32R = mybir.dt.float32r
BF16 = mybir.dt.bfloat16
AX = mybir.AxisListType
ALU = mybir.AluOpType
ACTF = mybir.ActivationFunctionType

N_CORES = 8
IPC = 4  # images per core
EPS = 1e-12
BF = ml_dtypes.bfloat16


def _round_f32r(a):
    u = np.ascontiguousarray(a, np.float32).view(np.uint32)
    u = (u + 0x200) & np.uint32(0xFFFFFC00)
    return u.view(np.float32)


def _build_xband(x):
    """x [N,3,512,512] f32 -> xband [N, 128, 17, 514] bf16.

    Tile r covers conv1 outputs [32r-2, 32r+30); partition p = a*3+ci holds
    input row h = 32r-3+a at w-offset 1 (cols 0/513 are zero pad)."""
    N = x.shape[0]
    xb = np.zeros((N, 128, 17, 514), dtype=BF)
    xbf = x.astype(BF)
    for r in range(17):
        h0 = 32 * r - 3
        a_lo = max(0, -h0)
        a_hi = min(34, 512 - h0)
        for ci in range(3):
            ps = np.arange(a_lo, a_hi) * 3 + ci
            xb[:, ps, r, 1:513] = xbf[:, ci, h0 + a_lo:h0 + a_hi, :]
    return xb


def _build_consts(conv1_w, conv2_w, assign_w, assign_b, lin_w, lin_b):
    c1w = np.asarray(conv1_w, np.float32)
    c2w = np.asarray(conv2_w, np.float32)
    # conv1 banded lhsT (bf16, padded to 128 rows): rows p = a*3 + ci
    # (a = h_in - (32r-3), 0..33), cols q = j*4 + co (j = h_out - (32r-2)).
    # variants: 0 = mid, 1 = first tile (outputs h<0 masked), 2 = tail
    # (outputs h>=512 masked). Input-side zeros come from the host pad.
    W1 = np.zeros((128, 9, 128), np.float32)
    for dx in range(3):
        for co in range(4):
            for ci in range(3):
                for dy in range(3):
                    for j in range(32):
                        W1[(j + dy) * 3 + ci, dx, j * 4 + co] = c1w[co, ci, dy, dx]
    W1[:, 3:6, :] = W1[:, 0:3, :]
    W1[:, 3:6, 0:8] = 0.0        # outputs h=-2,-1
    W1[:, 6:9, :] = W1[:, 0:3, :]
    W1[:, 6:9, 8:128] = 0.0      # outputs h>=512
    # conv2 banded lhsT with pool1-h fold; w-pools stored as SUMs: 0.25 scale
    W2 = np.zeros((80, 3, 128), np.float32)
    for dx in range(3):
        for co in range(16):
            for ci in range(4):
                for dy in range(3):
                    for rr in range(8):
                        for half in range(2):
                            W2[(2 * rr + 2 * dy + half) * 4 + ci, dx, rr * 16 + co] = (
                                0.25 * c2w[co, ci, dy, dx]
                            )
    # combined SA rhs [128, 128]: cols 0:16 logits (rows p = rrel*16 + c,
    # col q*4 + k, pool2-h fold, 0.25 = pool2 avg); cols 16:80 the pair-sum
    # matrix S (xf^T accumulation): S[(rrel,c), 16 + q*16 + c] = 1 where
    # q = rrel // 2; cols 80:128 zero pad (PSUM bank alignment).
    aw = np.asarray(assign_w, np.float32)
    SA = np.zeros((128, 128), np.float32)
    for q in range(4):
        for k in range(4):
            for c in range(16):
                for half in range(2):
                    SA[(2 * q + half) * 16 + c, q * 4 + k] = 0.25 * aw[k, c]
    for rrel in range(8):
        for c in range(16):
            SA[rrel * 16 + c, 16 + (rrel // 2) * 16 + c] = 1.0
    brep = np.tile(np.asarray(assign_b, np.float32), 4).reshape(16)
    brep128 = np.broadcast_to(brep, (128, 16)).copy()
    return {
        "w1": W1.astype(BF),
        "w2": W2.astype(BF),
        "sa": SA.astype(BF),
        "brep": brep128.astype(np.float32),
        "cent": np.zeros(0),  # set by caller (4x centroids)
        "wlin": np.asarray(lin_w, np.float32).T.copy(),  # (64, 7)
        "linb": np.broadcast_to(
            np.asarray(lin_b, np.float32).reshape(1, 7), (4, 7)).copy(),
        "ones41": np.ones((4, 1), np.float32),
    }


def _build_program():
    nc = bacc.Bacc("TRN2", target_bir_lowering=False, debug=False,
                   num_devices=N_CORES)
    xb = nc.dram_tensor("xb", [IPC, 128, 17, 514], BF16, kind="ExternalInput").ap()
    w1 = nc.dram_tensor("w1", [128, 9, 128], BF16, kind="ExternalInput").ap()
    w2 = nc.dram_tensor("w2", [80, 3, 128], BF16, kind="ExternalInput").ap()
    sa = nc.dram_tensor("sa", [128, 128], BF16, kind="ExternalInput").ap()
    brep = nc.dram_tensor("brep", [128, 16], F32, kind="ExternalInput").ap()
    cent = nc.dram_tensor("cent", [4, 16], F32, kind="ExternalInput").ap()
    wlin = nc.dram_tensor("wlin", [64, 7], F32, kind="ExternalInput").ap()
    linb = nc.dram_tensor("linb", [4, 7], F32, kind="ExternalInput").ap()
    ones41 = nc.dram_tensor("ones41", [4, 1], F32, kind="ExternalInput").ap()
    out = nc.dram_tensor("out", [IPC, 7], F32, kind="ExternalOutput").ap()

    from contextlib import ExitStack

    with tile.TileContext(nc) as tc, ExitStack() as es:
        consts = es.enter_context(tc.tile_pool(name="consts", bufs=1))
        xbp = es.enter_context(tc.tile_pool(name="xbp", bufs=3))
        y1p = es.enter_context(tc.tile_pool(name="y1p", bufs=2))
        x2p = es.enter_context(tc.tile_pool(name="x2p", bufs=3))
        vp = es.enter_context(tc.tile_pool(name="vp", bufs=2))
        xftp = es.enter_context(tc.tile_pool(name="xftp", bufs=2))
        smp = es.enter_context(tc.tile_pool(name="smp", bufs=3))
        finp = es.enter_context(tc.tile_pool(name="finp", bufs=2))
        p1p = es.enter_context(tc.tile_pool(name="p1p", bufs=3, space="PSUM"))
        p2p = es.enter_context(tc.tile_pool(name="p2p", bufs=2, space="PSUM"))
        sap = es.enter_context(tc.tile_pool(name="sap", bufs=2, space="PSUM"))
        gramp = es.enter_context(tc.tile_pool(name="gramp", bufs=1, space="PSUM"))

        # image 0 loads first on the sync queue in 3 chunks (so conv1(0) can
        # start after ~1/3 of the transfer); consts go via scalar.
        xb0p = es.enter_context(tc.tile_pool(name="xb0p", bufs=1))
        xb0_chunks = []
        for ci, (lo, hi) in enumerate(((0, 6), (6, 12), (12, 17))):
            ch = xb0p.tile([128, hi - lo, 514], BF16, tag=f"xb0c{ci}",
                           name=f"xb0c{ci}")
            nc.sync.dma_start(out=ch, in_=xb[0, :, lo:hi, :])
            xb0_chunks.append((lo, hi, ch))
        xb_tiles = [None]
        w1_sb = consts.tile([128, 9, 128], BF16)
        nc.scalar.dma_start(out=w1_sb, in_=w1)
        # persistent softmax-weight tile: cols 4:32 of each q-block stay zero
        # forever; per-image softmax only rewrites cols 0:4.
        apad = consts.tile([128, 32, 4, 32], BF16)
        nc.vector.memset(apad, 0.0)
        w2_sb = consts.tile([80, 3, 128], BF16)
        nc.scalar.dma_start(out=w2_sb, in_=w2)
        sa_sb = consts.tile([128, 128], BF16)
        nc.scalar.dma_start(out=sa_sb, in_=sa)
        brep_sb = consts.tile([128, 16], F32)
        nc.scalar.dma_start(out=brep_sb, in_=brep)
        cent_sb = consts.tile([4, 16], F32)
        nc.scalar.dma_start(out=cent_sb, in_=cent)
        wlin_sb = consts.tile([64, 7], F32)
        nc.scalar.dma_start(out=wlin_sb, in_=wlin)
        linb_sb = consts.tile([4, 7], F32)
        nc.scalar.dma_start(out=linb_sb, in_=linb)
        ones41_sb = consts.tile([4, 1], F32)
        nc.scalar.dma_start(out=ones41_sb, in_=ones41)

        def emit_conv1(img):
            # prefetch the next image while this one computes
            if img + 1 < IPC:
                nxt = xbp.tile([128, 17, 514], BF16, tag="xb", name="xbn")
                nc.sync.dma_start(out=nxt, in_=xb[img + 1])
                xb_tiles.append(nxt)

            def rslice(r):
                if img == 0:
                    for lo, hi, ch in xb0_chunks:
                        if lo <= r < hi:
                            return ch[:, r - lo, :]
                return xb_tiles[img][:, r, :]

            y1 = y1p.tile([128, 17, 258], BF16, tag="y1")
            nc.vector.memset(y1[:, :, 0:1], 0.0)
            nc.vector.memset(y1[:, :, 257:258], 0.0)
            for r in range(17):
                var1 = 1 if r == 0 else (2 if r == 16 else 0)
                xr = rslice(r)
                p1 = p1p.tile([128, 512], F32, tag="p1")
                for dx in range(3):
                    nc.tensor.matmul(
                        p1, w1_sb[:, var1 * 3 + dx, :], xr[:, dx:dx + 512],
                        start=(dx == 0), stop=(dx == 2),
                    )
                p1v = p1.rearrange("p (w two) -> p w two", two=2)
                re1 = smp.tile([128, 256], F32, tag="re1")
                nc.scalar.activation(out=re1, in_=p1v[:, :, 0], func=ACTF.Relu)
                nc.vector.scalar_tensor_tensor(
                    out=y1[:, r, 1:257], in0=p1v[:, :, 1], scalar=0.0, in1=re1,
                    op0=ALU.max, op1=ALU.add,
                )
            return y1

        def emit_conv2_sa(img, y1):
            # conv2: even pairs read Y1 blocks directly; odd pairs are
            # staged with 2 SBUF DMAs each (window straddles two blocks).
            v = vp.tile([128, 32, 128], BF16, tag="v")
            vv = v.rearrange("p (b two) w -> p b two w", two=2)
            for pi in range(16):
                even = pi < 8
                if even:
                    b = 2 * pi
                    ts = (4 * pi, 4 * pi + 2)
                    vslc = vv[:, 2 * pi: 2 * pi + 2, 0, :]
                    rhs = y1[0:80, b: b + 2, :]
                else:
                    oi = pi - 8
                    ts = (4 * oi + 1, 4 * oi + 3)
                    vslc = vv[:, 2 * oi: 2 * oi + 2, 1, :]
                    x2 = x2p.tile([80, 2, 258], BF16, tag="x2")
                    for j in range(2):
                        t = ts[j]
                        b = t // 2
                        dmae = nc.sync if j == 0 else nc.scalar
                        dmae.dma_start(
                            out=x2[0:64, j, :], in_=y1[64:128, b, :])
                        dmae.dma_start(
                            out=x2[64:80, j, :], in_=y1[0:16, b + 1, :])
                    rhs = x2[:]
                p2 = p2p.tile([128, 2, 256], F32, tag="p2")
                for dx in range(3):
                    nc.tensor.matmul(
                        p2, w2_sb[:, dx, :], rhs[:, :, dx: dx + 256],
                        start=(dx == 0), stop=(dx == 2),
                    )
                p2v = p2.rearrange("p a (w two) -> p a w two", two=2)
                re2 = smp.tile([128, 2, 128], F32, tag="re2")
                nc.scalar.activation(
                    out=re2, in_=p2v[:, :, :, 0], func=ACTF.Relu)
                nc.vector.scalar_tensor_tensor(
                    out=vslc, in0=p2v[:, :, :, 1], scalar=0.0,
                    in1=re2, op0=ALU.max, op1=ALU.add,
                )
            # NetVLAD front: per tile ONE matmul (stationary V, rhs [A|S]):
            # out [w, 0:16] = logits^T, [w, 16:80] = pair-summed xf^T.
            xft = xftp.tile([128, 32, 4, 17], BF16, tag="xft")
            nc.vector.memset(xft[:, :, :, 16:17], 1.0)
            lb = xftp.tile([128, 32, 16], F32, tag="lb")
            for g in range(8):
                sa_ps = sap.tile([128, 4, 80], F32, tag="sa")
                for j in range(4):
                    t = 4 * g + j
                    nc.tensor.matmul(
                        sa_ps[:, j, :], v[:, t, :], sa_sb[:, 0:80],
                        start=True, stop=True,
                    )
                nc.vector.tensor_add(
                    lb[:, 4 * g: 4 * g + 4, :], sa_ps[:, :, 0:16],
                    brep_sb[:].unsqueeze(1).broadcast_to((128, 4, 16)))
                nc.scalar.activation(
                    out=xft[:, 4 * g: 4 * g + 4, :, 0:16],
                    in_=sa_ps[:, :, 16:80].rearrange("w a (q c) -> w a q c", q=4),
                    func=ACTF.Copy)
            return lb, xft

        gsbs = consts.tile([4, IPC, 17], F32, name="gsbs")

        def emit_tail(img, lb, xft):
            # batched softmax; chunked for the last image so the gram can
            # start before the whole softmax finishes (pipeline drain).
            chunks = ((0, 16), (16, 32)) if img == IPC - 1 else ((0, 32),)
            g32 = gramp.tile([128, 68], F32, tag="gfin")
            for ci, (lo, hi) in enumerate(chunks):
                n = hi - lo
                lbv = lb[:, lo:hi, :].rearrange("w t (q k) -> w t q k", k=4)
                mx = smp.tile([128, n, 4], F32, tag="mx")
                nc.vector.reduce_max(mx, lbv, axis=AX.X)
                ls = smp.tile([128, n, 4, 4], F32, tag="ls")
                nc.vector.tensor_sub(
                    ls, lbv, mx.unsqueeze(-1).broadcast_to((128, n, 4, 4)))
                ae = smp.tile([128, n, 4, 4], F32, tag="ae")
                nc.scalar.activation(out=ae, in_=ls, func=ACTF.Exp)
                zs = smp.tile([128, n, 4], F32, tag="zs")
                nc.vector.reduce_sum(zs, ae, axis=AX.X)
                rz = smp.tile([128, n, 4], F32, tag="rz")
                nc.vector.reciprocal(rz, zs)
                nc.vector.scalar_tensor_tensor(
                    out=apad[:, lo:hi, :, 0:4], in0=ae, scalar=0.25,
                    in1=rz.unsqueeze(-1).broadcast_to((128, n, 4, 4)),
                    op0=ALU.mult, op1=ALU.mult,
                )
                # gram: stationary = per-tile apad [w, (q,32)]
                for t in range(lo, hi):
                    nc.tensor.matmul(
                        g32, apad[:, t, :, :].rearrange("p a b -> p (a b)"),
                        xft[:, t, :, :].rearrange("p a b -> p (a b)"),
                        start=(t == 0), stop=(t == 31),
                    )
            # fold the diagonal blocks into gsbs[:, img, :]
            t0_ = finp.tile([4, 17], F32, tag="t0")
            nc.vector.tensor_copy(t0_, g32[0:4, 0:17])
            t1_ = finp.tile([4, 17], F32, tag="t1")
            nc.vector.tensor_add(t1_, t0_, g32[32:36, 17:34])
            t2_ = finp.tile([4, 17], F32, tag="t2")
            nc.vector.tensor_add(t2_, t1_, g32[64:68, 34:51])
            nc.vector.tensor_add(gsbs[:, img, :], t2_, g32[96:100, 51:68])

        def emit_final():
            # batched finale over all IPC images: gsbs [4, IPC, 17]
            cb = finp.tile([4, IPC, 16], F32, tag="cb")
            nc.vector.tensor_mul(
                cb, cent_sb[:].unsqueeze(1).broadcast_to((4, IPC, 16)),
                gsbs[:, :, 16:17].broadcast_to((4, IPC, 16)))
            v4 = finp.tile([4, IPC, 16], F32, tag="v4")
            nc.vector.tensor_sub(v4, gsbs[:, :, 0:16], cb)
            sq = finp.tile([4, IPC, 16], F32, tag="sq")
            nc.vector.tensor_mul(sq, v4, v4)
            rs = finp.tile([4, IPC, 1], F32, tag="rs")
            nc.vector.reduce_sum(rs, sq, axis=AX.X)
            # 1/max(sqrt(x), 1e-12) == exp(-0.5*ln(max(x, 1e-24))): keeps the
            # ACT engine on the ln/exp table (no Sqrt table swap).
            rsm = finp.tile([4, IPC, 1], F32, tag="rsm")
            nc.vector.tensor_scalar_max(rsm, rs, EPS * EPS)
            lnr = finp.tile([4, IPC, 1], F32, tag="lnr")
            nc.scalar.activation(out=lnr, in_=rsm, func=ACTF.Ln)
            rn = finp.tile([4, IPC, 1], F32, tag="rn")
            nc.scalar.activation(out=rn, in_=lnr, func=ACTF.Exp, scale=-0.5)
            vn = finp.tile([4, IPC, 16], F32, tag="vn")
            nc.vector.tensor_mul(vn, v4, rn.broadcast_to((4, IPC, 16)))
            sqn = finp.tile([4, IPC, 16], F32, tag="sqn")
            nc.vector.tensor_mul(sqn, vn, vn)
            rs2 = finp.tile([4, IPC, 1], F32, tag="rs2")
            nc.vector.reduce_sum(rs2, sqn, axis=AX.X)
            # per-image global norm^2: contract the 4 partitions on the PE
            tps = gramp.tile([IPC, 1], F32, tag="gfin")
            nc.tensor.matmul(
                tps, rs2.rearrange("p i one -> p (i one)"), ones41_sb[:],
                start=True, stop=True)
            tpm = finp.tile([IPC, 1], F32, tag="tpm")
            nc.vector.tensor_scalar_max(tpm, tps, EPS * EPS)
            lng = finp.tile([IPC, 1], F32, tag="lng")
            nc.scalar.activation(out=lng, in_=tpm, func=ACTF.Ln)
            g2 = finp.tile([IPC, 1], F32, tag="g2")
            nc.scalar.activation(out=g2, in_=lng, func=ACTF.Exp, scale=-0.5)
            vcol = finp.tile([64, IPC], F32, tag="vcol")
            for i in range(IPC):
                nc.sync.dma_start(out=vcol[:, i:i + 1], in_=vn[:, i, :])
            fps = gramp.tile([IPC, 7], F32, tag="gfin")
            nc.tensor.matmul(fps, vcol[:], wlin_sb[:], start=True, stop=True)
            osb = finp.tile([IPC, 7], F32, tag="osb")
            nc.vector.scalar_tensor_tensor(
                out=osb, in0=fps, scalar=g2[:], in1=linb_sb[:],
                op0=ALU.mult, op1=ALU.add,
            )
            nc.sync.dma_start(out=out, in_=osb)

        # software pipeline: image i's softmax/gram runs while image i+1's
        # conv1 keeps the PE busy; one batched finale at the end.
        y1 = emit_conv1(0)
        lb, xft = emit_conv2_sa(0, y1)
        for img in range(1, IPC):
            y1 = emit_conv1(img)
            emit_tail(img - 1, lb, xft)
            lb, xft = emit_conv2_sa(img, y1)
        emit_tail(IPC - 1, lb, xft)
        emit_final()

    nc.compile()
    return nc


_CACHE = {}


def kernel(x, conv1_w, conv1_b, conv2_w, conv2_b, centroids, assign_w,
           assign_b, lin_w, lin_b):
    assert np.abs(np.asarray(conv1_b)).max() == 0.0
    assert np.abs(np.asarray(conv2_b)).max() == 0.0

    if "nc" not in _CACHE:
        _CACHE["nc"] = _build_program()
    nc = _CACHE["nc"]

    consts = _build_consts(conv1_w, conv2_w, assign_w, assign_b, lin_w, lin_b)
    consts["cent"] = 4.0 * np.asarray(centroids, np.float32)
    xband = _build_xband(np.asarray(x, np.float32))

    in_maps = []
    for c in range(N_CORES):
        m = dict(consts)
        m["xb"] = np.ascontiguousarray(xband[c * IPC: (c + 1) * IPC])
        in_maps.append(m)
    res = run_bass_kernel_spmd(nc, in_maps, list(range(N_CORES))).results
    return np.concatenate([res[c]["out"] for c in range(N_CORES)], axis=0)


if __name__ == "__main__":
    print("smoke test: building program only")
    _build_program()
    print("ok")
